# revision 13
# baseline (speedup 1.0000x reference)
"""Trainium2 Bass kernel for a full MHA transformer block.

Reference computation (per batch element, data-parallel over 8 NeuronCores):
    qh/kh/vh = (x @ W + b) split into 16 heads of 64
    attn     = softmax(qh @ kh^T / sqrt(64))
    ctx      = attn @ vh
    out      = LayerNorm(gelu(ctx @ Wo + bo) + residual) * gamma + beta

Shapes: B=8, S=1024, D=1024, H=16, DH=64, fp32.

v2 layout strategy (per core):
  - All PE matmul operands in bf16 (fp32r and bf16 both run 1 row/cycle on
    TRN2, but bf16 halves SBUF so weights can double-buffer and es can run
    deep; numerics sim: 1.8e-3 final rel err vs 2e-2 gate).
  - X^T via PE transposes with a bf16 identity (cost keys on the moving
    identity dtype: 1 cy/row), batched 4 per PSUM bank so one [128,512]
    DVE cast-copy drains 4 transposes.
  - Projection order k -> q -> v, with the first scores+exp groups emitted
    inside the v-projection window so ScalarE (the exp engine, the
    attention-phase floor) starts early.
  - Attention qh-outer so ct[:, :, qh=0] completes halfway; out-projection
    chunks interleave into the second attention half, filling PE gaps in
    the exp-bound steady state.
  - gelu/LayerNorm deferred to one batched tail (exp and gelu live in
    different ScalarE activation tables; interleaving would thrash
    ACT_TABLE_LOADs).
"""

import numpy as np

S, D, H, DH = 1024, 1024, 16, 64
EPS = 1e-5
NCORES = 8
P = 128
SC = S // P    # seq chunks (8)
DC = D // P    # feature chunks (8)
HP = H // 2    # head pairs (8)

_cache = {}


def _build(flags, debug=False):
    from contextlib import ExitStack

    import concourse.bass as bass
    import concourse.mybir as mybir
    import concourse.tile as tile
    from concourse import bacc
    from concourse.masks import make_identity

    f32 = mybir.dt.float32
    f32r = mybir.dt.float32r
    bf16 = mybir.dt.bfloat16
    AF = mybir.ActivationFunctionType
    Alu = mybir.AluOpType

    use_bq, use_bk, use_bv, use_bo, use_gam, use_bet = flags

    nc = bacc.Bacc(None, target_bir_lowering=False)

    xq = nc.dram_tensor("xq", [S, D], bf16, kind="ExternalInput")
    xk = nc.dram_tensor("xk", [S, D], bf16, kind="ExternalInput")
    xv = nc.dram_tensor("xv", [S, D], bf16, kind="ExternalInput")
    xres = nc.dram_tensor("xres", [S, D], f32, kind="ExternalInput")
    wq = nc.dram_tensor("wq", [D, D], bf16, kind="ExternalInput")
    wk = nc.dram_tensor("wk", [D, D], bf16, kind="ExternalInput")
    wv = nc.dram_tensor("wv", [D, D], bf16, kind="ExternalInput")
    wo = nc.dram_tensor("wo", [D, D], bf16, kind="ExternalInput")
    bq = nc.dram_tensor("bq", [D], f32, kind="ExternalInput")
    bk = nc.dram_tensor("bk", [D], f32, kind="ExternalInput")
    bv = nc.dram_tensor("bv", [D], f32, kind="ExternalInput")
    bo = nc.dram_tensor("bo", [D], f32, kind="ExternalInput")
    gam = nc.dram_tensor("gam", [D], f32, kind="ExternalInput")
    bet = nc.dram_tensor("bet", [D], f32, kind="ExternalInput")
    out = nc.dram_tensor("out", [S, D], f32, kind="ExternalOutput")
    if debug:
        d_qt = nc.dram_tensor("d_qt", [P, DC, S], f32, kind="ExternalOutput")
        d_kt = nc.dram_tensor("d_kt", [P, DC, S], f32, kind="ExternalOutput")
        d_vx = nc.dram_tensor("d_vx", [P, SC, H, DH + 1], f32, kind="ExternalOutput")
        d_ct = nc.dram_tensor("d_ct", [P, DC, S], f32, kind="ExternalOutput")

    with tile.TileContext(nc) as tc, ExitStack() as top:
        consts = top.enter_context(tc.tile_pool(name="consts", bufs=1))
        bigp = top.enter_context(tc.tile_pool(name="bigp", bufs=1))
        wp = top.enter_context(tc.tile_pool(name="wp", bufs=2))
        esp2 = None
        qkvp = top.enter_context(tc.tile_pool(name="qkvp", bufs=1))
        xnp = top.enter_context(tc.tile_pool(name="xnp", bufs=3))
        esp = top.enter_context(tc.tile_pool(name="esp", bufs=14))
        rcp = top.enter_context(tc.tile_pool(name="rcp", bufs=1))
        tmpp = top.enter_context(tc.tile_pool(name="tmp", bufs=2))
        yp = top.enter_context(tc.tile_pool(name="yp", bufs=8))
        xn2 = top.enter_context(tc.tile_pool(name="xn2", bufs=3))
        stp = top.enter_context(tc.tile_pool(name="stp", bufs=4))
        mvp = top.enter_context(tc.tile_pool(name="mvp", bufs=1))
        # PSUM (8 banks): proj phase uses pa (4x [P,512]) + ps_s (2x [P,1024],
        # eager scores); attention phase closes pa and opens ps_c (2) + pop (2)
        ps_s = top.enter_context(tc.tile_pool(name="ps_s", bufs=2, space="PSUM"))
        pool_box = {}

        ident = consts.tile([P, P], bf16, tag="ident")
        make_identity(nc, ident[:])

        need_ones = use_bv or use_bo
        if need_ones:
            ones1 = consts.tile([1, P], bf16, tag="ones1")
            nc.vector.memset(ones1[:], 1.0)
        if use_bq:
            bq_sb = consts.tile([P, DC], f32, tag="bq")
            nc.sync.dma_start(out=bq_sb[:], in_=bq[:].rearrange("(c p) -> p c", p=P))
        if use_bk:
            bk_sb = consts.tile([P, DC], f32, tag="bk")
            nc.sync.dma_start(out=bk_sb[:], in_=bk[:].rearrange("(c p) -> p c", p=P))
        if use_bv:
            bv_sb = consts.tile([1, D], bf16, tag="bv")
            bv_f = consts.tile([1, D], f32, tag="bvf")
            nc.sync.dma_start(out=bv_f[:], in_=bv[:].rearrange("d -> 1 d"))
            nc.vector.tensor_copy(bv_sb[:], bv_f[:])
        if use_bo:
            bo_sb = consts.tile([1, D], bf16, tag="bo")
            bo_f = consts.tile([1, D], f32, tag="bof")
            nc.sync.dma_start(out=bo_f[:], in_=bo[:].rearrange("d -> 1 d"))
            nc.vector.tensor_copy(bo_sb[:], bo_f[:])
        if use_gam:
            gam_bc = consts.tile([P, D], f32, tag="gam")
            nc.sync.dma_start(
                out=gam_bc[:],
                in_=bass.AP(tensor=gam[:].tensor, offset=0, ap=[[0, P], [1, D]]),
            )
        if use_bet:
            bet_bc = consts.tile([P, D], f32, tag="bet")
            nc.sync.dma_start(
                out=bet_bc[:],
                in_=bass.AP(tensor=bet[:].tensor, offset=0, ap=[[0, P], [1, D]]),
            )
        eps_sb = consts.tile([P, 1], f32, tag="eps")
        nc.vector.memset(eps_sb[:], EPS)

        def load_w(wd, name):
            # weights staged bf16 in DRAM: one strided DMA on the SWDGE
            # queue (keeps the Sync queue free for xn chunks / dn rows)
            w_sb = wp.tile([P, DC, D], bf16, tag="w", name=name)
            nc.gpsimd.dma_start(
                out=w_sb[:], in_=wd[:].rearrange("(c p) d -> p c d", p=P)
            )
            return w_sb

        qt = qkvp.tile([P, DC, S], bf16, tag="qt")
        kt = qkvp.tile([P, DC, S], bf16, tag="kt")
        vx = qkvp.tile([P, SC, H, DH + 1], bf16, tag="vx")
        xt = bigp.tile([P, DC, S], bf16, tag="xt")
        ct = bigp.tile([P, DC, S], bf16, tag="ct")
        ones16 = consts.tile([P, H], f32, tag="ones16")
        nc.vector.memset(ones16[:], 1.0)
        for sc in range(SC):
            nc.vector.tensor_copy(vx[:, sc, :, DH], ones16[:])

        # ---------------- projections ----------------
        def transpose_in(xd):
            # X [S, D] (DRAM fp32) -> xt [P, DC, S] bf16 (X^T), 4 transposes
            # per PSUM bank, one [128,512] DVE cast-copy per bank fill
            for sp in range(SC // 2):
                xn = xnp.tile([P, 2, D], bf16, tag="xn")
                eng = nc.sync if sp % 2 == 0 else nc.gpsimd
                eng.dma_start(
                    out=xn[:],
                    in_=xd[sp * 256:(sp + 1) * 256, :].rearrange(
                        "(c p) d -> p c d", p=P
                    ),
                )
                for c in range(2):
                    sc = 2 * sp + c
                    pt = pool_box["pa"].tile([P, D], bf16, tag="pa", name="tp")
                    for kc in range(DC):
                        nc.tensor.transpose(
                            pt[:, kc * P:(kc + 1) * P],
                            xn[:, c, kc * P:(kc + 1) * P],
                            ident[:],
                        )
                    dst = xt[:, :, sc * P:(sc + 1) * P]
                    src = pt[:].rearrange("p (c q) -> p c q", q=P)
                    nc.vector.tensor_copy(dst, src)

        def project_T(w_sb, dst, bias_sb, on_group=None):
            # dst[p, mc, s] = (X @ W)[s, mc*128+p] (+ bias); sh outer so the
            # first query half of every head completes early
            for sh in range(2):
                ssl = slice(sh * 512, (sh + 1) * 512)
                for mc0 in range(0, DC, 2):
                    psA = pool_box["pa"].tile([P, 512], f32, tag="pa", name="psA")
                    psB = pool_box["pa"].tile([P, 512], f32, tag="pa", name="psB")
                    for kc in range(DC):
                        for ps, mc in ((psA, mc0), (psB, mc0 + 1)):
                            nc.tensor.matmul(
                                ps[:],
                                w_sb[:, kc, mc * P:(mc + 1) * P],
                                xt[:, kc, ssl],
                                start=(kc == 0),
                                stop=(kc == DC - 1),
                            )
                    for i, (ps, mc) in enumerate(((psA, mc0), (psB, mc0 + 1))):
                        d = dst[:, mc, ssl]
                        if bias_sb is not None:
                            nc.vector.tensor_scalar_add(
                                d, in0=ps[:], scalar1=bias_sb[:, mc:mc + 1]
                            )
                        else:
                            nc.vector.tensor_copy(d, ps[:])
                    if on_group is not None:
                        on_group(sh, mc0)

        def project_V_chunk(w_sb, sc):
            # vx[p, sc, h, d] = (Xv @ Wv)[sc*128+p, h*64+d] (+ bias)
            psA = pool_box["pa"].tile([P, 512], f32, tag="pa", name="psA")
            psB = pool_box["pa"].tile([P, 512], f32, tag="pa", name="psB")
            for kc in range(DC):
                for ps, nh in ((psA, 0), (psB, 1)):
                    nc.tensor.matmul(
                        ps[:],
                        xt[:, kc, sc * P:(sc + 1) * P],
                        w_sb[:, kc, nh * 512:(nh + 1) * 512],
                        start=(kc == 0),
                        stop=(kc == DC - 1) and not use_bv,
                    )
            if use_bv:
                for ps, nh in ((psA, 0), (psB, 1)):
                    nc.tensor.matmul(
                        ps[:],
                        ones1[:],
                        bv_sb[0:1, nh * 512:(nh + 1) * 512],
                        start=False,
                        stop=True,
                    )
            for ps, nh in ((psA, 0), (psB, 1)):
                dst = vx[:, sc, nh * 8:(nh + 1) * 8, 0:DH]
                srcp = ps[:].rearrange("p (h d) -> p h d", d=DH)
                nc.vector.tensor_copy(dst, srcp)

        # ---------------- attention pieces ----------------
        es_store = {}   # (hp_i, qh, kt_i) -> es tile

        def emit_scores(hp_i, qh, kt_i):
            qsl = slice(qh * 512, (qh + 1) * 512)
            ks = slice(kt_i * P, (kt_i + 1) * P)
            ps = ps_s.tile([P, 1024], f32, tag="ps")
            nc.tensor.matmul(
                ps[:, 0:512],
                kt[0:64, hp_i, ks],
                qt[0:64, hp_i, qsl],
                start=True, stop=True,
                tile_position=(0, 0),
            )
            nc.tensor.matmul(
                ps[:, 512:1024],
                kt[64:128, hp_i, ks],
                qt[64:128, hp_i, qsl],
                start=True, stop=True,
                tile_position=(64, 0),
            )
            es = esp.tile([P, 1024], bf16, tag="es")
            nc.scalar.activation(es[:], ps[:], AF.Exp, scale=0.125)
            es_store[(hp_i, qh, kt_i)] = es

        pending_norm = []

        def emit_normalize(hp_i, qh, pcA, pcB):
            # ctx^T = uctx^T * (1/denom) broadcast
            qsl = slice(qh * 512, (qh + 1) * 512)
            hA, hB = 2 * hp_i, 2 * hp_i + 1
            for h, pc in ((hA, pcA), (hB, pcB)):
                dn = rcp.tile([DH + 1, 512], f32, tag="dn")
                nc.vector.tensor_copy(dn[DH:DH + 1, :], pc[DH:DH + 1, :])
                dn0 = rcp.tile([1, 512], f32, tag="dn0")
                nc.sync.dma_start(out=dn0[:], in_=dn[DH:DH + 1, :])
                rbc = rcp.tile([DH, 512], f32, tag="rbc")
                nc.gpsimd.partition_broadcast(rbc[:], dn0[:])
                nc.vector.reciprocal_approx_fast(out=rbc[:], in_=rbc[:])
                if h % 2 == 0:
                    nc.vector.tensor_mul(
                        ct[0:64, hp_i, qsl], pc[0:DH, :], rbc[:]
                    )
                else:
                    tmp = tmpp.tile([DH, 512], bf16, tag="tmp")
                    nc.vector.tensor_mul(tmp[:], pc[0:DH, :], rbc[:])
                    nc.sync.dma_start(
                        out=ct[64:128, hp_i, qsl], in_=tmp[:]
                    )

        def flush_norm():
            while pending_norm:
                emit_normalize(*pending_norm.pop(0))

        def emit_group(hp_i, qh):
            # ctx accumulation for one (head pair, query half); scores run
            # 4 k-tiles ahead of uctx, and the PREVIOUS group's normalize is
            # emitted behind the first 4 scores so its drain latency hides
            # under this group's scores/exp instead of stalling the PE queue
            hA, hB = 2 * hp_i, 2 * hp_i + 1
            pcA = pool_box["pc"].tile([DH + 1, 512], f32, tag="pc")
            pcB = pool_box["pc"].tile([DH + 1, 512], f32, tag="pc")

            def emit_uctx(kt_i):
                es = es_store.pop((hp_i, qh, kt_i))
                nc.tensor.matmul(
                    pcA[:],
                    vx[:, kt_i, hA, :],
                    es[:, 0:512],
                    start=(kt_i == 0), stop=(kt_i == SC - 1),
                )
                nc.tensor.matmul(
                    pcB[:],
                    vx[:, kt_i, hB, :],
                    es[:, 512:1024],
                    start=(kt_i == 0), stop=(kt_i == SC - 1),
                )

            flush_norm()
            for kt_i in range(SC + 6):
                if kt_i < SC and (hp_i, qh, kt_i) not in es_store:
                    emit_scores(hp_i, qh, kt_i)
                if kt_i >= 6:
                    emit_uctx(kt_i - 6)
            pending_norm.append((hp_i, qh, pcA, pcB))

        # ---------------- output projection pieces ----------------
        mv_all = mvp.tile([P, SC, 2], f32, tag="mv")
        rstd = mvp.tile([P, SC], f32, tag="rstd")
        y_tiles = [None] * SC
        xn_tiles = [None] * SC
        wo_sb_holder = [None]

        def emit_out_chunk(sc, fuse_gelu=False):
            # matmuls + PSUM drain; tail chunks fuse gelu into the drain
            # (they run after the last exp, so no table thrash)
            wo_sb = wo_sb_holder[0]
            ssl = slice(sc * P, (sc + 1) * P)
            y = yp.tile([P, D], f32, tag="y")
            y_tiles[sc] = y
            for nh in range(2):
                po = pool_box["po"].tile([P, 512], f32, tag="po")
                for mc in range(DC):
                    nc.tensor.matmul(
                        po[:],
                        ct[:, mc, ssl],
                        wo_sb[:, mc, nh * 512:(nh + 1) * 512],
                        start=(mc == 0),
                        stop=(mc == DC - 1) and not use_bo,
                    )
                if use_bo:
                    nc.tensor.matmul(
                        po[:],
                        ones1[:],
                        bo_sb[0:1, nh * 512:(nh + 1) * 512],
                        start=False, stop=True,
                    )
                if fuse_gelu:
                    nc.scalar.activation(
                        y[:, nh * 512:(nh + 1) * 512], po[:], AF.Gelu
                    )
                else:
                    nc.vector.tensor_copy(y[:, nh * 512:(nh + 1) * 512], po[:])

        def emit_out_finish(sc, fused=False):
            # gelu (batched table unless fused into the drain), residual, stats
            y = y_tiles[sc]
            xn = xn2.tile([P, D], f32, tag="xn2")
            nc.sync.dma_start(out=xn[:], in_=xres[sc * P:(sc + 1) * P, :])
            xn_tiles[sc] = xn
            if not fused:
                nc.scalar.activation(y[:], y[:], AF.Gelu)
            nc.vector.tensor_add(y[:], y[:], xn[:])
            st = stp.tile([P, 2, 6], f32, tag="st")
            nc.vector.bn_stats(st[:, 0, :], y[:, 0:512])
            nc.vector.bn_stats(st[:, 1, :], y[:, 512:1024])
            nc.vector.bn_aggr(mv_all[:, sc, :], st[:])

        def emit_finalize(batch):
            bsl = slice(batch[0], batch[-1] + 1)
            nc.scalar.activation(
                rstd[:, bsl], mv_all[:, bsl, 1], AF.Sqrt, bias=eps_sb[:]
            )
            nc.vector.reciprocal(rstd[:, bsl], rstd[:, bsl])
            for sc in batch:
                y = y_tiles[sc]
                nc.vector.tensor_scalar(
                    out=y[:],
                    in0=y[:],
                    scalar1=mv_all[:, sc, 0:1],
                    scalar2=rstd[:, sc:sc + 1],
                    op0=Alu.subtract,
                    op1=Alu.mult,
                )
                if use_gam:
                    nc.vector.tensor_mul(y[:], y[:], gam_bc[:])
                if use_bet:
                    nc.vector.tensor_add(y[:], y[:], bet_bc[:])
                nc.sync.dma_start(out=out[sc * P:(sc + 1) * P, :], in_=y[:])
                y_tiles[sc] = None
                xn_tiles[sc] = None

        # ---------------- emission schedule ----------------
        eager = [(hp, 0, k) for hp in range(2) for k in range(SC)][:14]

        def feed_eager(n=1):
            for _ in range(n):
                if eager:
                    emit_scores(*eager.pop(0))

        with tc.tile_pool(name="pa", bufs=4, space="PSUM") as pa:
            pool_box["pa"] = pa
            with nc.named_scope("proj_k"):
                transpose_in(xk)
                w_sb = load_w(wk, "wk")
                project_T(w_sb, kt, bk_sb if use_bk else None)
            with nc.named_scope("proj_q"):
                transpose_in(xq)
                w_sb = load_w(wq, "wq")
                # qh=0 scores need only the sh=0 half of qt: feed eager
                # scores into the sh=1 groups
                project_T(
                    w_sb, qt, bq_sb if use_bq else None,
                    on_group=lambda sh, mc0: feed_eager(2 if sh == 1 else 0),
                )
            with nc.named_scope("proj_v"):
                transpose_in(xv)
                w_sb = load_w(wv, "wv")
                for sc in range(SC):
                    project_V_chunk(w_sb, sc)
                    feed_eager(1)
        wo_sb_holder[0] = load_w(wo, "wo")

        with tc.tile_pool(name="pc", bufs=2, space="PSUM") as pcp, \
             tc.tile_pool(name="po", bufs=2, space="PSUM") as pop:
            pool_box["pc"] = pcp
            pool_box["po"] = pop
            with nc.named_scope("attention"):
                # qh outer: ct[:, :, qh=0] completes after first 8 groups;
                # out chunks interleave into the second half
                for qh in range(2):
                    for hp_i in range(HP):
                        emit_group(hp_i, qh)
                        if qh == 1 and hp_i >= 1 and (hp_i - 1) % 2 == 0:
                            emit_out_chunk((hp_i - 1) // 2)
            with nc.named_scope("out_proj"):
                flush_norm()
                if debug:
                    nc.sync.dma_start(out=d_qt[:], in_=qt[:].bitcast(f32))
                    nc.sync.dma_start(out=d_kt[:], in_=kt[:].bitcast(f32))
                    nc.sync.dma_start(out=d_vx[:], in_=vx[:].bitcast(f32))
                    nc.sync.dma_start(out=d_ct[:], in_=ct[:].bitcast(f32))
                # keep the gelu/LN tail behind the exps in scheduler order
                # (mixing them thrashes ScalarE activation tables)
                with tc.high_priority(offset=-1000000):
                    for sc in range(4, SC):
                        emit_out_chunk(sc, fuse_gelu=True)
                    for sc in range(SC):
                        emit_out_finish(sc, fused=(sc >= 4))
                    emit_finalize([0, 1, 2])
                    emit_finalize([3, 4, 5])
                    emit_finalize([6, 7])

    nc.finalize()
    return nc


def _get_nc(flags):
    if flags not in _cache:
        _cache[flags] = _build(flags)
    return _cache[flags]


def kernel(q, k, v, wq, bq, wk, bk, wv, bv, wo, bo, ln_gamma, ln_beta):
    import ml_dtypes
    from concourse.bass_utils import run_bass_kernel_spmd

    bf = ml_dtypes.bfloat16
    kernel_inputs_q = q
    q = np.ascontiguousarray(q).astype(bf)
    k = np.ascontiguousarray(k).astype(bf)
    v = np.ascontiguousarray(v).astype(bf)

    flags = (
        bool(np.any(bq)), bool(np.any(bk)), bool(np.any(bv)), bool(np.any(bo)),
        not bool(np.all(ln_gamma == 1.0)), bool(np.any(ln_beta)),
    )
    nc = _get_nc(flags)

    shared = {
        "wq": np.ascontiguousarray(wq).astype(bf),
        "wk": np.ascontiguousarray(wk).astype(bf),
        "wv": np.ascontiguousarray(wv).astype(bf),
        "wo": np.ascontiguousarray(wo).astype(bf),
        "bq": np.ascontiguousarray(bq, np.float32),
        "bk": np.ascontiguousarray(bk, np.float32),
        "bv": np.ascontiguousarray(bv, np.float32),
        "bo": np.ascontiguousarray(bo, np.float32),
        "gam": np.ascontiguousarray(ln_gamma, np.float32),
        "bet": np.ascontiguousarray(ln_beta, np.float32),
    }
    qf = np.ascontiguousarray(kernel_inputs_q, np.float32)
    in_maps = [
        {"xq": q[b], "xk": k[b], "xv": v[b], "xres": qf[b], **shared}
        for b in range(NCORES)
    ]
    res = run_bass_kernel_spmd(nc, in_maps, core_ids=list(range(NCORES)))
    return np.stack([res.results[b]["out"] for b in range(NCORES)], axis=0)
